# revision 5
# baseline (speedup 1.0000x reference)
"""Trainium2 Bass kernel for GridMultiAtomLoss.

Data-parallel over the batch dim: 64 items -> 8 NeuronCores x 8 items.
Each core computes, for its 8 items:
  - sse of the distance grids (the memory-bound part: 2 x 10.5 MB streamed)
  - argmin over ref atoms, masked CE on class logits, masked BCE on edges
and writes per-item partials; the host concatenates and takes the means.

Self-contained: hardcodes shapes/sharding; only needs /opt/trn_rl_repo.
"""

import sys
from contextlib import ExitStack

import numpy as np

for _p in ("/opt/trn_rl_repo",):
    if _p not in sys.path:
        sys.path.insert(0, _p)

import concourse.bacc as bacc
import concourse.tile as tile
from concourse import mybir
from concourse.bass_utils import run_bass_kernel_spmd

AF = mybir.ActivationFunctionType
ALU = mybir.AluOpType
AX = mybir.AxisListType
FP32 = mybir.dt.float32
I32 = mybir.dt.int32
U32 = mybir.dt.uint32

B = 64          # global batch
CORES = 8
S = B // CORES  # items per core
N = 54          # ref atoms
E = 54          # edges
C = 5           # classes
GRID = 20 * 128 * 128          # elements per item grid
LOG_CLAMP = -100.0

# grid stream chunking: (start elem, n elems, psum-selector row)
# rows of the selector matmul: 0 = sum over partitions<64, 1 = partitions>=64,
# 2 = all partitions.  2-item chunks put item a in rows 0..63 and item a+1 in
# rows 64..127; within-one-item chunks use the full-sum row.  Tapered sizes so
# the post-DMA sub+square tail is short.
CHUNKS = [
    (0 * GRID, 2 * GRID, "half"),
    (2 * GRID, 2 * GRID, "half"),
    (4 * GRID, 2 * GRID, "half"),
    (6 * GRID, 1 * GRID, "one"),
    (7 * GRID, GRID // 2, "one"),
    (7 * GRID + GRID // 2, GRID // 2, "one"),
]
NCH = len(CHUNKS)

_NC = None


def _body(ctx, tc, aps):
    nc = tc.nc
    (pred_nodes, pred_edges, pred_dist, ref_nodes, ref_edges, ref_term,
     ref_dist, out_mse, out_ce, out_bce, out_idx) = aps

    gridp = ctx.enter_context(tc.tile_pool(name="grid_pred", bufs=3))
    gridr = ctx.enter_context(tc.tile_pool(name="grid_ref", bufs=3))
    small = ctx.enter_context(tc.tile_pool(name="small", bufs=1))
    psum = ctx.enter_context(tc.tile_pool(name="psum", bufs=1, space="PSUM"))

    # ---------------- small input DMAs (first: highest priority) ----------------
    nodes = small.tile([S, N * 4], FP32)
    nc.sync.dma_start(nodes[:], ref_nodes.rearrange("b n c -> b (n c)"))
    pnodes = small.tile([S, 3 + C], FP32)
    nc.sync.dma_start(pnodes[:], pred_nodes)
    pedges = small.tile([S, E], FP32)
    nc.sync.dma_start(pedges[:], pred_edges)
    edges = small.tile([S, N * E], FP32)
    nc.sync.dma_start(edges[:], ref_edges.rearrange("b n e -> b (n e)"))
    term = small.tile([S, 1], I32)
    nc.sync.dma_start(term[:], ref_term.unsqueeze(1))

    # ---------------- grid sse stream (memory bound) ----------------
    pred_flat = pred_dist.rearrange("b c h w -> (b c h w)")
    ref_flat = ref_dist.rearrange("b c h w -> (b c h w)")

    acc = small.tile([128, NCH], FP32)
    for j, (start, nelem, _) in enumerate(CHUNKS):
        f = nelem // 128
        pt = gridp.tile([128, f], FP32, tag="pred")
        rt = gridr.tile([128, f], FP32, tag="ref")
        nc.sync.dma_start(
            pt[:], pred_flat[start : start + nelem].rearrange("(p f) -> p f", p=128)
        )
        nc.scalar.dma_start(
            rt[:], ref_flat[start : start + nelem].rearrange("(p f) -> p f", p=128)
        )
        # diff in place on the pred tile, square+row-sum on ScalarE
        nc.vector.tensor_sub(pt[:], pt[:], rt[:])
        nc.scalar.activation(rt[:], pt[:], AF.Square, accum_out=acc[:, j : j + 1])

    # selector matmul: rows (lo half, hi half, all) x chunk columns
    sel3 = small.tile([128, 3], FP32)
    nc.vector.memset(sel3[:], 0.0)
    nc.vector.memset(sel3[0:64, 0:1], 1.0)
    nc.vector.memset(sel3[64:128, 1:2], 1.0)
    nc.vector.memset(sel3[:, 2:3], 1.0)
    psse = psum.tile([3, NCH], FP32)
    nc.tensor.matmul(psse[:], sel3[:], acc[:], start=True, stop=True)
    mse_sb = small.tile([3, NCH], FP32)
    nc.scalar.copy(mse_sb[:], psse[:])
    nc.sync.dma_start(out_mse.rearrange("(r j) -> r j", r=3), mse_sb[:])

    # ---------------- small per-item path ----------------
    nodes_v = nodes[:].rearrange("b (n c) -> b n c", c=4)

    # squared distance to predicted position, negated for arg-min-via-max
    d2 = small.tile([S, N], FP32)
    dd = small.tile([S, N], FP32)
    for c in range(3):
        dc = small.tile([S, N], FP32, tag=f"dc{c}")
        nc.vector.tensor_single_scalar(
            dc[:], nodes_v[:, :, c], pnodes[:, c : c + 1], ALU.subtract
        )
        if c == 0:
            nc.vector.tensor_mul(d2[:], dc[:], dc[:])
        else:
            nc.vector.tensor_mul(dd[:], dc[:], dc[:])
            nc.vector.tensor_add(d2[:], d2[:], dd[:])
    nd2 = small.tile([S, N], FP32)
    nc.vector.tensor_scalar_mul(nd2[:], d2[:], -1.0)

    mx8 = small.tile([S, 8], FP32)
    idx8 = small.tile([S, 8], U32)
    nc.vector.max(mx8[:], nd2[:])
    nc.vector.max_index(idx8[:], mx8[:], nd2[:])
    minind_f = small.tile([S, 1], FP32)
    nc.vector.tensor_copy(minind_f[:], idx8[:, 0:1])
    nc.sync.dma_start(out_idx.unsqueeze(1), idx8[:, 0:1].bitcast(I32))

    # one-hot mask over atoms for the matched atom
    iota_i = small.tile([S, N], I32)
    nc.gpsimd.iota(iota_i[:], pattern=[[1, N]], base=0, channel_multiplier=0)
    iota_f = small.tile([S, N], FP32)
    nc.vector.tensor_copy(iota_f[:], iota_i[:])
    mask = small.tile([S, N], FP32)
    nc.vector.tensor_single_scalar(mask[:], iota_f[:], minind_f[:], ALU.is_equal)

    # matched class index (exact small ints in f32)
    tgt = small.tile([S, 1], FP32)
    junk_n = small.tile([S, N], FP32)
    nc.vector.tensor_mul(junk_n[:], mask[:], nodes_v[:, :, 3])
    nc.vector.tensor_reduce(tgt[:], junk_n[:], axis=AX.X, op=ALU.add)

    # active = (ref_terminate == 0)
    termf = small.tile([S, 1], FP32)
    nc.vector.tensor_copy(termf[:], term[:])
    active = small.tile([S, 1], FP32)
    nc.vector.tensor_single_scalar(active[:], termf[:], 0.0, ALU.is_equal)

    # ----- cross entropy on logits -----
    logits = pnodes[:, 3 : 3 + C]
    m = small.tile([S, 1], FP32)
    nc.vector.tensor_reduce(m[:], logits, axis=AX.X, op=ALU.max)
    negm = small.tile([S, 1], FP32)
    nc.vector.tensor_scalar_mul(negm[:], m[:], -1.0)
    expt = small.tile([S, C], FP32)
    sumexp = small.tile([S, 1], FP32)
    nc.scalar.activation(expt[:], logits, AF.Exp, bias=negm[:], scale=1.0,
                         accum_out=sumexp[:])
    lse = small.tile([S, 1], FP32)
    nc.scalar.activation(lse[:], sumexp[:], AF.Ln)
    maskc = small.tile([S, C], FP32)
    nc.vector.tensor_single_scalar(maskc[:], iota_f[:, 0:C], tgt[:], ALU.is_equal)
    tlogit = small.tile([S, 1], FP32)
    junk_c = small.tile([S, C], FP32)
    nc.vector.tensor_mul(junk_c[:], maskc[:], logits)
    nc.vector.tensor_reduce(tlogit[:], junk_c[:], axis=AX.X, op=ALU.add)
    ce1 = small.tile([S, 1], FP32)
    nc.vector.tensor_add(ce1[:], m[:], lse[:])
    ce2 = small.tile([S, 1], FP32)
    nc.vector.tensor_sub(ce2[:], ce1[:], tlogit[:])
    nllc = small.tile([S, 1], FP32)
    nc.vector.tensor_mul(nllc[:], ce2[:], active[:])
    nc.sync.dma_start(out_ce.unsqueeze(1), nllc[:])

    # ----- BCE on edge probabilities vs the matched ref edge row -----
    lp = small.tile([S, E], FP32)
    nc.scalar.activation(lp[:], pedges[:], AF.Ln)
    nc.vector.tensor_scalar_max(lp[:], lp[:], LOG_CLAMP)
    l1p = small.tile([S, E], FP32)
    nc.scalar.activation(l1p[:], pedges[:], AF.Ln, bias=1.0, scale=-1.0)
    nc.vector.tensor_scalar_max(l1p[:], l1p[:], LOG_CLAMP)
    w = small.tile([S, E], FP32)
    nc.vector.tensor_sub(w[:], lp[:], l1p[:])
    sum_l1p = small.tile([S, 1], FP32)
    nc.vector.tensor_reduce(sum_l1p[:], l1p[:], axis=AX.X, op=ALU.add)

    # sel_edges[b, e] = sum_n mask[b, n] * ref_edges[b, n, e]
    masked = small.tile([S, N * E], FP32)
    edges_v = edges[:].rearrange("b (n e) -> b n e", e=E)
    mask_b = mask[:].unsqueeze(2).broadcast_to([S, N, E])
    nc.vector.tensor_mul(masked[:].rearrange("b (n e) -> b n e", e=E), edges_v, mask_b)
    sel = small.tile([S, E], FP32)
    nc.vector.tensor_reduce(
        sel[:], masked[:].rearrange("b (n e) -> b e n", e=E), axis=AX.X, op=ALU.add
    )
    dot = small.tile([S, 1], FP32)
    junk_e = small.tile([S, E], FP32)
    nc.vector.tensor_mul(junk_e[:], sel[:], w[:])
    nc.vector.tensor_reduce(dot[:], junk_e[:], axis=AX.X, op=ALU.add)
    bsum = small.tile([S, 1], FP32)
    nc.vector.tensor_add(bsum[:], dot[:], sum_l1p[:])
    bce = small.tile([S, 1], FP32)
    nc.vector.tensor_scalar_mul(bce[:], bsum[:], -1.0 / E)
    nllb = small.tile([S, 1], FP32)
    nc.vector.tensor_mul(nllb[:], bce[:], active[:])
    nc.sync.dma_start(out_bce.unsqueeze(1), nllb[:])


def build():
    nc = bacc.Bacc("TRN2", target_bir_lowering=False, debug=False,
                   enable_asserts=False, num_devices=CORES)
    aps = (
        nc.dram_tensor("pred_nodes", [S, 3 + C], FP32, kind="ExternalInput").ap(),
        nc.dram_tensor("pred_edges", [S, E], FP32, kind="ExternalInput").ap(),
        nc.dram_tensor("pred_dist", [S, 20, 128, 128], FP32, kind="ExternalInput").ap(),
        nc.dram_tensor("ref_nodes", [S, N, 4], FP32, kind="ExternalInput").ap(),
        nc.dram_tensor("ref_edges", [S, N, E], FP32, kind="ExternalInput").ap(),
        nc.dram_tensor("ref_terminate", [S], I32, kind="ExternalInput").ap(),
        nc.dram_tensor("ref_dist", [S, 20, 128, 128], FP32, kind="ExternalInput").ap(),
        nc.dram_tensor("out_mse", [3 * NCH], FP32, kind="ExternalOutput").ap(),
        nc.dram_tensor("out_ce", [S], FP32, kind="ExternalOutput").ap(),
        nc.dram_tensor("out_bce", [S], FP32, kind="ExternalOutput").ap(),
        nc.dram_tensor("out_minind", [S], I32, kind="ExternalOutput").ap(),
    )
    with tile.TileContext(nc) as tc:
        with ExitStack() as ctx:
            _body(ctx, tc, aps)
    nc.compile()
    return nc


def get_nc():
    global _NC
    if _NC is None:
        _NC = build()
    return _NC


def decode_mse(raw):
    """raw: [3*NCH] selector-matmul output -> per-item mse [S]."""
    r = np.asarray(raw, np.float64).reshape(3, NCH)
    sse = np.zeros(S)
    sse[0], sse[1] = r[0, 0], r[1, 0]
    sse[2], sse[3] = r[0, 1], r[1, 1]
    sse[4], sse[5] = r[0, 2], r[1, 2]
    sse[6] = r[2, 3]
    sse[7] = r[2, 4] + r[2, 5]
    return sse / GRID


def shard_inputs(inputs):
    in_maps = []
    for c in range(CORES):
        sl = slice(c * S, (c + 1) * S)
        in_maps.append(
            {
                "pred_nodes": np.ascontiguousarray(inputs["pred_nodes"][sl], np.float32),
                "pred_edges": np.ascontiguousarray(inputs["pred_edges"][sl], np.float32),
                "pred_dist": np.ascontiguousarray(inputs["pred_dist"][sl], np.float32),
                "ref_nodes": np.ascontiguousarray(inputs["ref_nodes"][sl], np.float32),
                "ref_edges": np.ascontiguousarray(inputs["ref_edges"][sl], np.float32),
                "ref_terminate": np.ascontiguousarray(inputs["ref_terminate"][sl], np.int32),
                "ref_dist": np.ascontiguousarray(inputs["ref_dist"][sl], np.float32),
            }
        )
    return in_maps


def combine(results):
    mse = np.concatenate([decode_mse(r["out_mse"]) for r in results])
    ce = np.concatenate([np.asarray(r["out_ce"]) for r in results])
    bce = np.concatenate([np.asarray(r["out_bce"]) for r in results])
    idx = np.concatenate([np.asarray(r["out_minind"]) for r in results]).astype(np.int32)
    loss = np.float32(
        np.float64(mse.mean()) + np.float64(ce.mean()) + np.float64(bce.mean())
    )
    return loss, idx


def kernel(**inputs):
    nc = get_nc()
    in_maps = shard_inputs({k: np.asarray(v) for k, v in inputs.items()})
    res = run_bass_kernel_spmd(nc, in_maps, list(range(CORES))).results
    return combine(res)
